# revision 1
# baseline (speedup 1.0000x reference)
"""Trainium2 Bass kernel: C = Au @ Bu for packed upper-triangular Au, Bu.

Inputs (full): A, B — packed row-major upper-triangular storage of two
512x512 f32 matrices, each a flat array of length 131328 = 512*513/2.
Output: dense [512, 512] f32 C = unpack(A) @ unpack(B)  (upper triangular).

Strategy — balanced triangular brick decomposition:
  C is tiled into [128, 128] bricks.  Brick (R, nb) only needs
  contraction k-blocks kt in [R, nb] (A is upper-tri -> k >= 128R;
  B is upper-tri -> k <= 128nb+127), so of the 64 (brick, kt) products
  only 20 are nonzero.  Those 20 MM bricks are spread over the 8 cores
  (3 slots each, zero-padded), every brick an independent
  [128k x 128m] @ [128k x 128n] native-fp32 PE matmul.  Bricks of the
  same (R, nb) land on PSUM/host as partial sums and are added during
  unsharding (host add; error ~1 ulp vs a single fp32 accumulation).

  Per core: 3x 128KB input chunks (one per brick, own semaphore so each
  matmul starts at its minimal dependency), 3 matmuls into 3 PSUM
  tensors, per-brick DVE copy and per-brick output DMA so the store
  pipeline drains while later bricks still compute.  Raw bacc program
  (no Tile ceremony); the entry const-AP memsets and exit all-engine
  barrier (unneeded here) are stripped from the IR.
"""

import numpy as np

N = 512
P = 128
KT = 4  # k-blocks in the full problem
NCORES = 8
S = 3  # brick slots per core
BW = 256  # slab cols per slot: A part 128 + B part 128
PACKED_LEN = N * (N + 1) // 2

# core -> (slot0, slot1, slot2); slot = (R, nb, kt) brick coordinates
# (C rows 128R.., cols 128nb.., contraction k-block kt), or None for a
# zero-padded slot.
ASSIGN = [
    ((0, 3, 0), (0, 3, 1), (0, 3, 2)),
    ((0, 3, 3), (1, 3, 1), (1, 3, 2)),
    ((1, 3, 3), (2, 3, 2), (2, 3, 3)),
    ((3, 3, 3), (0, 2, 0), (0, 2, 1)),
    ((0, 2, 2), (1, 2, 1), (1, 2, 2)),
    ((2, 2, 2), (0, 1, 0), (0, 1, 1)),
    ((1, 1, 1), (0, 0, 0), None),
    (None, None, None),
]
# C brick (R, nb) -> list of (core, slot) contributions to sum.
BRICK_SRC = {}
for _g, _slots in enumerate(ASSIGN):
    for _s, _u in enumerate(_slots):
        if _u is not None:
            BRICK_SRC.setdefault((_u[0], _u[1]), []).append((_g, _s))

_CACHE = {}


def _unpack_upper(p):
    """Packed row-major upper-tri -> dense [N, N] with zero lower triangle."""
    p = np.asarray(p, dtype=np.float32).reshape(-1)
    i = np.arange(N)[:, None]
    j = np.arange(N)[None, :]
    mask = j >= i
    pidx = np.where(mask, (i * (2 * N - i + 1)) // 2 + (j - i), 0)
    return np.where(mask, p[pidx], np.float32(0.0))


def _strip_framework_ceremony(nc):
    """IR surgery on the built program:
    - drop the 4 unused const-AP memsets in the entry block (they gate
      the entry all-engine barrier on the Pool engine by ~400ns);
    - drop the exit all-engine barrier EventSemaphores (the final SP
      wait_ge(osem) already guarantees the output landed; per-engine
      drains are kept);
    - hoist the three input DMACopies to the head of the entry block,
      ahead of SP's entry-barrier participation, so descriptor
      generation and the transfers overlap the barrier (~300ns).  Safe:
      nothing reads the SBUF tile before its per-chunk semaphore fires,
      and the runtime resets semaphores between executions (verified by
      repeat runs)."""
    import concourse.mybir as mybir

    f = nc.m.functions[0]
    entry = f.blocks[0]
    entry.instructions = [
        i
        for i in entry.instructions
        if not (
            isinstance(i, mybir.InstMemset)
            and i.outs
            and "const-" in str(getattr(i.outs[0].bass_ap.tensor, "name", ""))
        )
    ]
    for bb in f.blocks:
        if bb.name.endswith("_end"):
            bb.instructions = [
                i
                for i in bb.instructions
                if not (
                    isinstance(i, mybir.InstEventSemaphore)
                    and str(i.name).startswith("aeb_barrier")
                )
            ]
    moved = []
    for bb in f.blocks:
        dmas = [
            i
            for i in bb.instructions
            if isinstance(i, mybir.InstDMACopy)
            and i.outs
            and "t_" in str(getattr(i.outs[0].bass_ap.tensor, "name", ""))
        ]
        if dmas:
            bb.instructions = [i for i in bb.instructions if i not in dmas]
            moved += dmas
    entry.instructions = moved + entry.instructions


def _build_nc():
    import concourse.mybir as mybir
    from concourse import bacc

    F32 = mybir.dt.float32

    nc = bacc.Bacc("TRN2", num_devices=NCORES)
    ab = nc.dram_tensor("ab", [P, S, BW], F32, kind="ExternalInput")
    cdr = nc.dram_tensor("c", [P, S, 128], F32, kind="ExternalOutput")

    with (
        nc.sbuf_tensor([P, S, BW], F32) as t,
        nc.sbuf_tensor([P, S, 128], F32) as ostage,
        # One PSUM tensor per slot: independent accumulation groups, and
        # each slot's copy must not read another slot's open group.
        nc.psum_tensor([P, 128], F32) as ps0,
        nc.psum_tensor([P, 128], F32) as ps1,
        nc.psum_tensor([P, 128], F32) as ps2,
        # One semaphore per input chunk: DMAs sharing one sem could
        # interleave per-engine completions, so a cumulative wait wouldn't
        # prove an individual chunk landed (CoreSim race detector).
        nc.semaphore("ds0") as ds0,
        nc.semaphore("ds1") as ds1,
        nc.semaphore("ds2") as ds2,
        nc.semaphore("osem") as osem,
        nc.semaphore("osem_p") as osem_p,
        nc.semaphore("psem") as psem,
        nc.semaphore("vsem") as vsem,
        nc.Block(no_gpsimd_drain=True) as block,
    ):
        dsems = [ds0, ds1, ds2]
        psums = [ps0, ps1, ps2]

        # Slot 1's input chunk and output store ride the SWDGE (Pool)
        # path: Q7 descriptor generation runs in parallel with the HWDGE
        # chain, so chunk 1's transfer slots between chunks 0/2 (earlier
        # receipts for MM1/MM2) and out 1 stays off the HWDGE chain that
        # gates out 2.  SWDGE requires its semaphore to start from 0, so
        # the Pool store ticks its own osem_p.
        @block.sync
        def _(sync):
            for s in (0, 2):
                sync.dma_start(out=t.ap()[:, s], in_=ab.ap()[:, s]).then_inc(
                    dsems[s], 16
                )
            # Per-brick stores: slot s's output DMA launches as soon as its
            # copy lands, overlapping later bricks' matmuls/copies.
            for s in (0, 2):
                sync.wait_ge(vsem, s + 1)
                sync.dma_start(
                    out=cdr.ap()[:, s], in_=ostage.ap()[:, s]
                ).then_inc(osem, 16)
            # Wait on the later-completing Pool store first so the
            # already-satisfied HWDGE wait costs no extra sequencer time.
            sync.wait_ge(osem_p, 16)
            sync.wait_ge(osem, 32)

        @block.gpsimd
        def _(gp):
            gp.dma_start(out=t.ap()[:, 1], in_=ab.ap()[:, 1]).then_inc(
                dsems[1], 16
            )
            gp.wait_ge(vsem, 2)
            gp.dma_start(out=cdr.ap()[:, 1], in_=ostage.ap()[:, 1]).then_inc(
                osem_p, 16
            )

        @block.tensor
        def _(tensor):
            for s in range(S):
                tensor.wait_ge(dsems[s], 16)
                nc.tensor.matmul(
                    psums[s].ap(),
                    t.ap()[:, s, :128],
                    t.ap()[:, s, 128:],
                    start=True,
                    stop=True,
                ).then_inc(psem, 1)

        @block.vector
        def _(vector):
            for s in range(S):
                vector.wait_ge(psem, s + 1)
                nc.vector.tensor_copy(
                    ostage.ap()[:, s], psums[s].ap()
                ).then_inc(vsem, 1)

    _strip_framework_ceremony(nc)
    nc.compile()
    return nc


def _get_nc():
    if "nc" not in _CACHE:
        _CACHE["nc"] = _build_nc()
    return _CACHE["nc"]


def _make_in_maps(A, B):
    Au = _unpack_upper(A)
    Bu = _unpack_upper(B)
    aT = np.ascontiguousarray(Au.T)  # aT[k, m] = Au[m, k]
    aTk = aT.reshape(KT, P, N)  # [kt, p, m]
    Buk = Bu.reshape(KT, P, N)  # [kt, p, n]
    in_maps = []
    for slots in ASSIGN:
        abarr = np.zeros((P, S, BW), dtype=np.float32)
        for s, unit in enumerate(slots):
            if unit is None:
                continue
            R, nb, kt = unit
            abarr[:, s, :128] = aTk[kt, :, R * P : (R + 1) * P]
            abarr[:, s, 128:] = Buk[kt, :, nb * P : (nb + 1) * P]
        in_maps.append({"ab": abarr})
    return in_maps


def _get_runner():
    """Build the sharded PJRT executable once; reuse across kernel() calls.

    Mirrors concourse.bass2jax.run_bass_via_pjrt's multi-core path, but
    caches the jitted function so repeat calls skip retracing.
    """
    if "runner" in _CACHE:
        return _CACHE["runner"]
    import jax
    import concourse.mybir as mybir
    from concourse import bass2jax
    from jax.experimental.shard_map import shard_map
    from jax.sharding import Mesh, PartitionSpec

    nc = _get_nc()
    bass2jax.install_neuronx_cc_hook()
    partition_name = (
        nc.partition_id_tensor.name if nc.partition_id_tensor else None
    )
    in_names, out_names, out_avals, zero_outs = [], [], [], []
    for alloc in nc.m.functions[0].allocations:
        if not isinstance(alloc, mybir.MemoryLocationSet):
            continue
        name = alloc.memorylocations[0].name
        if alloc.kind == "ExternalInput":
            if name != partition_name:
                in_names.append(name)
        elif alloc.kind == "ExternalOutput":
            out_names.append(name)
            shape = tuple(alloc.tensor_shape)
            dtype = mybir.dt.np(alloc.dtype)
            out_avals.append(jax.core.ShapedArray(shape, dtype))
            zero_outs.append(np.zeros(shape, dtype))
    n_params = len(in_names)
    n_outs = len(out_names)
    all_in = in_names + out_names + ([partition_name] if partition_name else [])
    donate = tuple(range(n_params, n_params + n_outs))

    def _body(*args):
        operands = list(args)
        if partition_name is not None:
            operands.append(bass2jax.partition_id_tensor())
        outs = bass2jax._bass_exec_p.bind(
            *operands,
            out_avals=tuple(out_avals),
            in_names=tuple(all_in),
            out_names=tuple(out_names),
            lowering_input_output_aliases=(),
            sim_require_finite=True,
            sim_require_nnan=True,
            nc=nc,
        )
        return tuple(outs)

    devices = jax.devices()[:NCORES]
    mesh = Mesh(np.asarray(devices), ("core",))
    fn = jax.jit(
        shard_map(
            _body,
            mesh=mesh,
            in_specs=(PartitionSpec("core"),) * (n_params + n_outs),
            out_specs=(PartitionSpec("core"),) * n_outs,
            check_rep=False,
        ),
        donate_argnums=donate,
        keep_unused=True,
    )
    runner = dict(
        fn=fn, in_names=in_names, out_names=out_names, zero_outs=zero_outs
    )
    _CACHE["runner"] = runner
    return runner


def _run_concat(concat_in):
    """Execute on 8 cores given axis-0-concatenated per-core inputs."""
    r = _get_runner()
    concat_zeros = [
        np.zeros((NCORES * z.shape[0], *z.shape[1:]), z.dtype)
        for z in r["zero_outs"]
    ]
    return r["fn"](*concat_in, *concat_zeros)


def _concat_inputs(in_maps):
    r = _get_runner()
    return [
        np.concatenate([in_maps[c][n] for c in range(NCORES)], axis=0)
        for n in r["in_names"]
    ]


def _assemble(out0):
    # out0: concat over cores of [P, S, 128] -> [NCORES, P(m), S, 128(n)]
    bricks = np.asarray(out0).reshape(NCORES, P, S, 128)
    C = np.zeros((N, N), dtype=np.float32)
    for (R, nb), srcs in BRICK_SRC.items():
        (g0, s0) = srcs[0]
        acc = bricks[g0, :, s0, :].copy()
        for g, s in srcs[1:]:
            acc += bricks[g, :, s, :]
        C[R * P : (R + 1) * P, nb * P : (nb + 1) * P] = acc
    return C


def kernel(A, B):
    in_maps = _make_in_maps(A, B)
    concat_in = _concat_inputs(in_maps)
    out = _run_concat(concat_in)
    return _assemble(out[0])



# revision 8
# speedup vs baseline: 23344.0465x; 23344.0465x over previous
"""Trainium2 Bass kernel: C = Au @ Bu for packed upper-triangular Au, Bu.

Inputs (full): A, B — packed row-major upper-triangular storage of two
512x512 f32 matrices, each a flat array of length 131328 = 512*513/2.
Output: dense [512, 512] f32 C = unpack(A) @ unpack(B)  (upper triangular).

Strategy — A-block grouping, bf16 PE, prepared-descriptor output:
  C bricks (R, nb) need contraction blocks kt in [R, nb]; grouping the 20
  nonzero (R, nb, kt) products by their stationary A^T block (R, kt) gives
  10 groups whose moving B columns are contiguous.  Each core runs a
  uniform program with two matmul slots (256 + 128 moving columns, 2560
  total columns over 8 cores), inputs converted to bf16 host-side
  (tolerance is 2e-2; bf16 rounding contributes ~2e-3).

  Per core: ONE input DMACopy ([128, 1280B] bf16, so the fixed
  HWDGE/DGE-delay prefix and the 900ns DMA-sem latency are paid once),
  two bf16 matmuls into separate PSUM banks, PSUM->SBUF staging copies
  split across Activation (256 cols) and DVE (128 cols), and the output
  store as an SWDGE scatter-add whose descriptors are PREPARED on the
  Pool engine during the input phase (identity indices built on-chip with
  iota/mod/add) and fired with trigger_dma as soon as staging lands —
  skipping the 625+650ns HWDGE path on the critical output tail.
  Partial C bricks are summed on the host during unsharding.
"""

import numpy as np

N = 512
P = 128
KT = 4
NCORES = 8
CA = 256  # slot-a moving columns
CB = 128  # slot-b moving columns
IN_W = 128 + 128 + CA + CB  # lhsT_a, lhsT_b, B_a, B_b  (bf16 elements)
OUT_W = CA + CB
PACKED_LEN = N * (N + 1) // 2

# core -> (slot_a, slot_b); slot_a = (R, kt, nb0) covering C cols
# [128*nb0, 128*nb0+256); slot_b = (R, kt, nb) covering one 128-col brick.
ASSIGN = [
    ((0, 0, 0), (0, 3, 3)),
    ((0, 0, 2), (1, 3, 3)),
    ((0, 1, 1), (0, 1, 3)),
    ((1, 1, 1), (1, 1, 3)),
    ((0, 2, 2), (2, 3, 3)),
    ((1, 2, 2), (3, 3, 3)),
    ((2, 2, 2), None),
    (None, None),
]

_CACHE = {}


def _unpack_upper(p):
    """Packed row-major upper-tri -> dense [N, N] with zero lower triangle."""
    p = np.asarray(p, dtype=np.float32).reshape(-1)
    i = np.arange(N)[:, None]
    j = np.arange(N)[None, :]
    mask = j >= i
    pidx = np.where(mask, (i * (2 * N - i + 1)) // 2 + (j - i), 0)
    return np.where(mask, p[pidx], np.float32(0.0))


def _strip_framework_ceremony(nc):
    """IR surgery on the built program (as in the tuned baseline):
    - drop unused const-AP memsets in the entry block;
    - drop the exit all-engine-barrier EventSemaphores (the final SP
      wait_ge(osem) already guarantees the output landed);
    - hoist the input DMACopy (dram tensor "ab") to the head of the entry
      block so descriptor generation overlaps the entry barrier."""
    import concourse.mybir as mybir

    f = nc.m.functions[0]
    entry = f.blocks[0]
    entry.instructions = [
        i
        for i in entry.instructions
        if not (
            isinstance(i, mybir.InstMemset)
            and i.outs
            and "const-" in str(getattr(i.outs[0].bass_ap.tensor, "name", ""))
        )
    ]
    for bb in f.blocks:
        if bb.name.endswith("_end"):
            bb.instructions = [
                i
                for i in bb.instructions
                if not (
                    isinstance(i, mybir.InstEventSemaphore)
                    and str(i.name).startswith("aeb_barrier")
                )
            ]
    moved = []
    for bb in f.blocks:
        dmas = [
            i
            for i in bb.instructions
            if isinstance(i, mybir.InstDMACopy)
            and i.ins
            and "ab" == str(getattr(i.ins[0].bass_ap.tensor, "name", ""))
        ]
        if dmas:
            bb.instructions = [i for i in bb.instructions if i not in dmas]
            moved += dmas
    entry.instructions = moved + entry.instructions


def _build_nc():
    import concourse.mybir as mybir
    from concourse import bacc

    F32 = mybir.dt.float32
    BF16 = mybir.dt.bfloat16
    I16 = mybir.dt.int16

    nc = bacc.Bacc("TRN2", num_devices=NCORES)
    ab = nc.dram_tensor("ab", [P, IN_W], BF16, kind="ExternalInput")
    cdr = nc.dram_tensor("c", [2 * P, OUT_W], BF16, kind="ExternalOutput")

    with (
        nc.sbuf_tensor([P, IN_W], BF16) as t,
        nc.sbuf_tensor([P, 1, OUT_W], BF16) as ostage,
        nc.sbuf_tensor([P, 8], I16) as idx,
        nc.psum_tensor([P, CA], F32) as psa,
        nc.psum_tensor([P, CB], F32) as psb,
        nc.semaphore("dsem") as dsem,
        nc.semaphore("psem") as psem,
        nc.semaphore("vsem") as vsem,
        nc.semaphore("prepsem") as prepsem,
        nc.semaphore("osem") as osem,
        nc.Block(no_gpsimd_drain=True) as block,
    ):

        @block.sync
        def _(sync):
            sync.dma_start(out=t.ap(), in_=ab.ap()).then_inc(dsem, 16)
            sync.wait_ge(osem, 16)

        @block.tensor
        def _(tensor):
            tensor.wait_ge(dsem, 16)
            nc.tensor.matmul(
                psa.ap(),
                t.ap()[:, 0:128],
                t.ap()[:, 256 : 256 + CA],
                start=True,
                stop=True,
            ).then_inc(psem, 1)
            nc.tensor.matmul(
                psb.ap(),
                t.ap()[:, 128:256],
                t.ap()[:, 256 + CA : IN_W],
                start=True,
                stop=True,
            ).then_inc(psem, 1)

        @block.scalar
        def _(scalar):
            # Staging split 224/32/128 across Act/DVE balances the two
            # engines' copy tails (Act has the larger fixed access latency).
            scalar.wait_ge(psem, 1)
            nc.scalar.copy(ostage.ap()[:, 0, 0:224], psa.ap()[:, 0:224]).then_inc(
                vsem, 1
            )

        @block.vector
        def _(vector):
            vector.wait_ge(psem, 1)
            nc.vector.tensor_copy(
                ostage.ap()[:, 0, 224:CA], psa.ap()[:, 224:CA]
            ).then_inc(vsem, 1)
            vector.wait_ge(psem, 2)
            nc.vector.tensor_copy(
                ostage.ap()[:, 0, CA:OUT_W], psb.ap()
            ).then_inc(vsem, 1)

        @block.gpsimd
        def _(gp):
            # Scatter indices: idx[p, s] = p + 16*s.  The descriptor
            # generator consumes the first 16 partitions, where this is the
            # identity permutation (idx i at partition i%16, column i//16).
            # The replica partitions hold shifted values; the output DRAM
            # tensor is padded to 2*P rows so they stay in-bounds, and the
            # host reads only the first P rows.
            gp.iota(idx.ap(), pattern=[[16, 8]], base=0, channel_multiplier=1)
            # Prepare the output store's descriptors now (Pool is otherwise
            # idle during the input phase); fire them the moment staging
            # lands.  dst row p receives ostage partition p.
            gp.dma_scatter_add(
                cdr.ap(),
                ostage.ap(),
                idx.ap(),
                num_idxs=P,
                num_idxs_reg=P,
                elem_size=OUT_W,
                prepare_only=True,
                sem=osem,
            ).then_inc(prepsem, 1)
            # Waits fused onto the trigger itself: the SEQ parks decoded,
            # firing the instant the last staging copy lands.
            gp.wait_ge(prepsem, 1)
            gp.trigger_dma(count=1)._wait_ge(vsem, 3)

    _strip_framework_ceremony(nc)
    nc.compile()
    return nc


def _get_nc():
    if "nc" not in _CACHE:
        _CACHE["nc"] = _build_nc()
    return _CACHE["nc"]


def _make_in_maps(A, B):
    import concourse.mybir as mybir

    bf16 = mybir.dt.np(mybir.dt.bfloat16)
    Au = _unpack_upper(A)
    Bu = _unpack_upper(B)
    aT = np.ascontiguousarray(Au.T)  # aT[k, m] = Au[m, k]
    aTk = aT.reshape(KT, P, N).astype(bf16)  # [kt, p, m]
    Buk = Bu.reshape(KT, P, N).astype(bf16)  # [kt, p, n]
    in_maps = []
    for sa, sb in ASSIGN:
        abarr = np.zeros((P, IN_W), dtype=bf16)
        if sa is not None:
            R, kt, nb0 = sa
            abarr[:, 0:128] = aTk[kt, :, R * P : (R + 1) * P]
            abarr[:, 256 : 256 + CA] = Buk[kt, :, nb0 * P : nb0 * P + CA]
        if sb is not None:
            R, kt, nb = sb
            abarr[:, 128:256] = aTk[kt, :, R * P : (R + 1) * P]
            abarr[:, 256 + CA : IN_W] = Buk[kt, :, nb * P : (nb + 1) * P]
        in_maps.append({"ab": abarr})
    return in_maps


def _get_runner():
    """Build the sharded PJRT executable once; reuse across kernel() calls."""
    if "runner" in _CACHE:
        return _CACHE["runner"]
    import jax
    import concourse.mybir as mybir
    from concourse import bass2jax
    from jax.experimental.shard_map import shard_map
    from jax.sharding import Mesh, PartitionSpec

    nc = _get_nc()
    bass2jax.install_neuronx_cc_hook()
    partition_name = (
        nc.partition_id_tensor.name if nc.partition_id_tensor else None
    )
    in_names, out_names, out_avals, zero_outs = [], [], [], []
    for alloc in nc.m.functions[0].allocations:
        if not isinstance(alloc, mybir.MemoryLocationSet):
            continue
        name = alloc.memorylocations[0].name
        if alloc.kind == "ExternalInput":
            if name != partition_name:
                in_names.append(name)
        elif alloc.kind == "ExternalOutput":
            out_names.append(name)
            shape = tuple(alloc.tensor_shape)
            dtype = mybir.dt.np(alloc.dtype)
            out_avals.append(jax.core.ShapedArray(shape, dtype))
            zero_outs.append(np.zeros(shape, dtype))
    n_params = len(in_names)
    n_outs = len(out_names)
    all_in = in_names + out_names + ([partition_name] if partition_name else [])
    donate = tuple(range(n_params, n_params + n_outs))

    def _body(*args):
        operands = list(args)
        if partition_name is not None:
            operands.append(bass2jax.partition_id_tensor())
        outs = bass2jax._bass_exec_p.bind(
            *operands,
            out_avals=tuple(out_avals),
            in_names=tuple(all_in),
            out_names=tuple(out_names),
            lowering_input_output_aliases=(),
            sim_require_finite=True,
            sim_require_nnan=True,
            nc=nc,
        )
        return tuple(outs)

    devices = jax.devices()[:NCORES]
    mesh = Mesh(np.asarray(devices), ("core",))
    fn = jax.jit(
        shard_map(
            _body,
            mesh=mesh,
            in_specs=(PartitionSpec("core"),) * (n_params + n_outs),
            out_specs=(PartitionSpec("core"),) * n_outs,
            check_rep=False,
        ),
        donate_argnums=donate,
        keep_unused=True,
    )
    runner = dict(
        fn=fn, in_names=in_names, out_names=out_names, zero_outs=zero_outs
    )
    _CACHE["runner"] = runner
    return runner


def _run_concat(concat_in):
    """Execute on 8 cores given axis-0-concatenated per-core inputs."""
    r = _get_runner()
    concat_zeros = [
        np.zeros((NCORES * z.shape[0], *z.shape[1:]), z.dtype)
        for z in r["zero_outs"]
    ]
    return r["fn"](*concat_in, *concat_zeros)


def _concat_inputs(in_maps):
    r = _get_runner()
    return [
        np.concatenate([in_maps[c][n] for c in range(NCORES)], axis=0)
        for n in r["in_names"]
    ]


def _assemble(out0):
    # out0: concat over cores of [P, OUT_W] f32 -> per-core partial bricks.
    outs = (
        np.asarray(out0)
        .astype(np.float32)
        .reshape(NCORES, 2 * P, OUT_W)[:, :P, :]
    )
    C = np.zeros((N, N), dtype=np.float32)
    for g, (sa, sb) in enumerate(ASSIGN):
        if sa is not None:
            R, kt, nb0 = sa
            C[R * P : (R + 1) * P, nb0 * P : nb0 * P + CA] += outs[g, :, 0:CA]
        if sb is not None:
            R, kt, nb = sb
            C[R * P : (R + 1) * P, nb * P : (nb + 1) * P] += outs[
                g, :, CA:OUT_W
            ]
    return C


def kernel(A, B):
    in_maps = _make_in_maps(A, B)
    concat_in = _concat_inputs(in_maps)
    out = _run_concat(concat_in)
    return _assemble(out[0])


# revision 10
# speedup vs baseline: 23897.2698x; 1.0237x over previous
"""Trainium2 Bass kernel: C = Au @ Bu for packed upper-triangular Au, Bu.

Inputs (full): A, B — packed row-major upper-triangular storage of two
512x512 f32 matrices, each a flat array of length 131328 = 512*513/2.
Output: dense [512, 512] f32 C = unpack(A) @ unpack(B)  (upper triangular).

Strategy — A-block grouping, bf16 PE, prepared-descriptor output:
  C bricks (R, nb) need contraction blocks kt in [R, nb]; grouping the 20
  nonzero (R, nb, kt) products by their stationary A^T block (R, kt) gives
  10 groups whose moving B columns are contiguous.  Each core runs a
  uniform program with two matmul slots (256 + 128 moving columns, 2560
  total columns over 8 cores), inputs converted to bf16 host-side
  (tolerance is 2e-2; bf16 rounding contributes ~2e-3).

  Per core: ONE input DMACopy ([128, 1280B] bf16, so the fixed
  HWDGE/DGE-delay prefix and the 900ns DMA-sem latency are paid once),
  two bf16 matmuls into separate PSUM banks, PSUM->SBUF bf16 staging
  copies split 224/32+128 across Activation/DVE (balances the engines'
  access-latency tails), and the output store as an SWDGE scatter-add
  whose descriptors are PREPARED on the Pool engine during the input
  phase (identity indices built on-chip with one iota) and fired with a
  wait-fused trigger_dma the instant staging lands — skipping the
  625+650ns HWDGE path on the critical output tail.  Output rides bf16
  (partials rounded once; ~1e-3 extra error) and is upcast and summed
  into C on the host during unsharding.  TimelineSim: 4726 ns.
"""

import numpy as np

N = 512
P = 128
KT = 4
NCORES = 8
CA = 256  # slot-a moving columns
CB = 128  # slot-b moving columns
IN_W = 128 + 128 + CA + CB  # lhsT_a, lhsT_b, B_a, B_b  (bf16 elements)
OUT_W = CA + CB
PACKED_LEN = N * (N + 1) // 2

# core -> (slot_a, slot_b); slot_a = (R, kt, nb0) covering C cols
# [128*nb0, 128*nb0+256); slot_b = (R, kt, nb) covering one 128-col brick.
ASSIGN = [
    ((0, 0, 0), (0, 3, 3)),
    ((0, 0, 2), (1, 3, 3)),
    ((0, 1, 1), (0, 1, 3)),
    ((1, 1, 1), (1, 1, 3)),
    ((0, 2, 2), (2, 3, 3)),
    ((1, 2, 2), (3, 3, 3)),
    ((2, 2, 2), None),
    (None, None),
]

_CACHE = {}


def _unpack_upper(p):
    """Packed row-major upper-tri -> dense [N, N] with zero lower triangle."""
    p = np.asarray(p, dtype=np.float32).reshape(-1)
    i = np.arange(N)[:, None]
    j = np.arange(N)[None, :]
    mask = j >= i
    pidx = np.where(mask, (i * (2 * N - i + 1)) // 2 + (j - i), 0)
    return np.where(mask, p[pidx], np.float32(0.0))


def _strip_framework_ceremony(nc):
    """IR surgery on the built program (as in the tuned baseline):
    - drop unused const-AP memsets in the entry block;
    - drop the exit all-engine-barrier EventSemaphores (the final SP
      wait_ge(osem) already guarantees the output landed);
    - hoist the input DMACopy (dram tensor "ab") to the head of the entry
      block so descriptor generation overlaps the entry barrier."""
    import concourse.mybir as mybir

    f = nc.m.functions[0]
    entry = f.blocks[0]
    entry.instructions = [
        i
        for i in entry.instructions
        if not (
            isinstance(i, mybir.InstMemset)
            and i.outs
            and "const-" in str(getattr(i.outs[0].bass_ap.tensor, "name", ""))
        )
    ]
    for bb in f.blocks:
        if bb.name.endswith("_end"):
            bb.instructions = [
                i
                for i in bb.instructions
                if not (
                    isinstance(i, mybir.InstEventSemaphore)
                    and str(i.name).startswith("aeb_barrier")
                )
                # SP's pipeline holds nothing (DMA + waits only); its final
                # osem wait is the completion guarantee, the drain is not.
                and not (
                    isinstance(i, mybir.InstDrain)
                    and i.engine == mybir.EngineType.SP
                )
            ]
    moved = []
    for bb in f.blocks:
        dmas = [
            i
            for i in bb.instructions
            if isinstance(i, mybir.InstDMACopy)
            and i.ins
            and "ab" == str(getattr(i.ins[0].bass_ap.tensor, "name", ""))
        ]
        if dmas:
            bb.instructions = [i for i in bb.instructions if i not in dmas]
            moved += dmas
    entry.instructions = moved + entry.instructions


def _build_nc():
    import concourse.mybir as mybir
    from concourse import bacc

    F32 = mybir.dt.float32
    BF16 = mybir.dt.bfloat16
    I16 = mybir.dt.int16

    nc = bacc.Bacc("TRN2", num_devices=NCORES)
    ab = nc.dram_tensor("ab", [P, IN_W], BF16, kind="ExternalInput")
    cdr = nc.dram_tensor("c", [2 * P, OUT_W], BF16, kind="ExternalOutput")

    with (
        nc.sbuf_tensor([P, IN_W], BF16) as t,
        nc.sbuf_tensor([P, 1, OUT_W], BF16) as ostage,
        nc.sbuf_tensor([P, 8], I16) as idx,
        nc.psum_tensor([P, CA], F32) as psa,
        nc.psum_tensor([P, CB], F32) as psb,
        nc.semaphore("dsem") as dsem,
        nc.semaphore("psem") as psem,
        nc.semaphore("vsem") as vsem,
        nc.semaphore("prepsem") as prepsem,
        nc.semaphore("osem") as osem,
        nc.Block(no_gpsimd_drain=True) as block,
    ):

        @block.sync
        def _(sync):
            sync.dma_start(out=t.ap(), in_=ab.ap()).then_inc(dsem, 16)
            sync.wait_ge(osem, 16)

        @block.tensor
        def _(tensor):
            tensor.wait_ge(dsem, 16)
            nc.tensor.matmul(
                psa.ap(),
                t.ap()[:, 0:128],
                t.ap()[:, 256 : 256 + CA],
                start=True,
                stop=True,
            ).then_inc(psem, 1)
            nc.tensor.matmul(
                psb.ap(),
                t.ap()[:, 128:256],
                t.ap()[:, 256 + CA : IN_W],
                start=True,
                stop=True,
            ).then_inc(psem, 1)

        @block.scalar
        def _(scalar):
            # Staging split 224/32/128 across Act/DVE balances the two
            # engines' copy tails (Act has the larger fixed access latency).
            scalar.wait_ge(psem, 1)
            nc.scalar.copy(ostage.ap()[:, 0, 0:224], psa.ap()[:, 0:224]).then_inc(
                vsem, 1
            )

        @block.vector
        def _(vector):
            vector.wait_ge(psem, 1)
            nc.vector.tensor_copy(
                ostage.ap()[:, 0, 224:CA], psa.ap()[:, 224:CA]
            ).then_inc(vsem, 1)
            vector.wait_ge(psem, 2)
            nc.vector.tensor_copy(
                ostage.ap()[:, 0, CA:OUT_W], psb.ap()
            ).then_inc(vsem, 1)

        @block.gpsimd
        def _(gp):
            # Scatter indices: idx[p, s] = p + 16*s.  The descriptor
            # generator consumes the first 16 partitions, where this is the
            # identity permutation (idx i at partition i%16, column i//16).
            # The replica partitions hold shifted values; the output DRAM
            # tensor is padded to 2*P rows so they stay in-bounds, and the
            # host reads only the first P rows.
            gp.iota(idx.ap(), pattern=[[16, 8]], base=0, channel_multiplier=1)
            # Prepare the output store's descriptors now (Pool is otherwise
            # idle during the input phase); fire them the moment staging
            # lands.  dst row p receives ostage partition p.
            gp.dma_scatter_add(
                cdr.ap(),
                ostage.ap(),
                idx.ap(),
                num_idxs=P,
                num_idxs_reg=P,
                elem_size=OUT_W,
                prepare_only=True,
                sem=osem,
            ).then_inc(prepsem, 1)
            # Waits fused onto the trigger itself: the SEQ parks decoded,
            # firing the instant the last staging copy lands.
            gp.wait_ge(prepsem, 1)
            gp.trigger_dma(count=1)._wait_ge(vsem, 3)

    _strip_framework_ceremony(nc)
    nc.compile()
    return nc


def _get_nc():
    if "nc" not in _CACHE:
        _CACHE["nc"] = _build_nc()
    return _CACHE["nc"]


def _make_in_maps(A, B):
    import concourse.mybir as mybir

    bf16 = mybir.dt.np(mybir.dt.bfloat16)
    Au = _unpack_upper(A)
    Bu = _unpack_upper(B)
    aT = np.ascontiguousarray(Au.T)  # aT[k, m] = Au[m, k]
    aTk = aT.reshape(KT, P, N).astype(bf16)  # [kt, p, m]
    Buk = Bu.reshape(KT, P, N).astype(bf16)  # [kt, p, n]
    in_maps = []
    for sa, sb in ASSIGN:
        abarr = np.zeros((P, IN_W), dtype=bf16)
        if sa is not None:
            R, kt, nb0 = sa
            abarr[:, 0:128] = aTk[kt, :, R * P : (R + 1) * P]
            abarr[:, 256 : 256 + CA] = Buk[kt, :, nb0 * P : nb0 * P + CA]
        if sb is not None:
            R, kt, nb = sb
            abarr[:, 128:256] = aTk[kt, :, R * P : (R + 1) * P]
            abarr[:, 256 + CA : IN_W] = Buk[kt, :, nb * P : (nb + 1) * P]
        in_maps.append({"ab": abarr})
    return in_maps


def _get_runner():
    """Build the sharded PJRT executable once; reuse across kernel() calls."""
    if "runner" in _CACHE:
        return _CACHE["runner"]
    import jax
    import concourse.mybir as mybir
    from concourse import bass2jax
    from jax.experimental.shard_map import shard_map
    from jax.sharding import Mesh, PartitionSpec

    nc = _get_nc()
    bass2jax.install_neuronx_cc_hook()
    partition_name = (
        nc.partition_id_tensor.name if nc.partition_id_tensor else None
    )
    in_names, out_names, out_avals, zero_outs = [], [], [], []
    for alloc in nc.m.functions[0].allocations:
        if not isinstance(alloc, mybir.MemoryLocationSet):
            continue
        name = alloc.memorylocations[0].name
        if alloc.kind == "ExternalInput":
            if name != partition_name:
                in_names.append(name)
        elif alloc.kind == "ExternalOutput":
            out_names.append(name)
            shape = tuple(alloc.tensor_shape)
            dtype = mybir.dt.np(alloc.dtype)
            out_avals.append(jax.core.ShapedArray(shape, dtype))
            zero_outs.append(np.zeros(shape, dtype))
    n_params = len(in_names)
    n_outs = len(out_names)
    all_in = in_names + out_names + ([partition_name] if partition_name else [])
    donate = tuple(range(n_params, n_params + n_outs))

    def _body(*args):
        operands = list(args)
        if partition_name is not None:
            operands.append(bass2jax.partition_id_tensor())
        outs = bass2jax._bass_exec_p.bind(
            *operands,
            out_avals=tuple(out_avals),
            in_names=tuple(all_in),
            out_names=tuple(out_names),
            lowering_input_output_aliases=(),
            sim_require_finite=True,
            sim_require_nnan=True,
            nc=nc,
        )
        return tuple(outs)

    devices = jax.devices()[:NCORES]
    mesh = Mesh(np.asarray(devices), ("core",))
    fn = jax.jit(
        shard_map(
            _body,
            mesh=mesh,
            in_specs=(PartitionSpec("core"),) * (n_params + n_outs),
            out_specs=(PartitionSpec("core"),) * n_outs,
            check_rep=False,
        ),
        donate_argnums=donate,
        keep_unused=True,
    )
    runner = dict(
        fn=fn, in_names=in_names, out_names=out_names, zero_outs=zero_outs
    )
    _CACHE["runner"] = runner
    return runner


def _run_concat(concat_in):
    """Execute on 8 cores given axis-0-concatenated per-core inputs."""
    r = _get_runner()
    concat_zeros = [
        np.zeros((NCORES * z.shape[0], *z.shape[1:]), z.dtype)
        for z in r["zero_outs"]
    ]
    return r["fn"](*concat_in, *concat_zeros)


def _concat_inputs(in_maps):
    r = _get_runner()
    return [
        np.concatenate([in_maps[c][n] for c in range(NCORES)], axis=0)
        for n in r["in_names"]
    ]


def _assemble(out0):
    # out0: concat over cores of [P, OUT_W] f32 -> per-core partial bricks.
    outs = (
        np.asarray(out0)
        .astype(np.float32)
        .reshape(NCORES, 2 * P, OUT_W)[:, :P, :]
    )
    C = np.zeros((N, N), dtype=np.float32)
    for g, (sa, sb) in enumerate(ASSIGN):
        if sa is not None:
            R, kt, nb0 = sa
            C[R * P : (R + 1) * P, nb0 * P : nb0 * P + CA] += outs[g, :, 0:CA]
        if sb is not None:
            R, kt, nb = sb
            C[R * P : (R + 1) * P, nb * P : (nb + 1) * P] += outs[
                g, :, CA:OUT_W
            ]
    return C


def kernel(A, B):
    in_maps = _make_in_maps(A, B)
    concat_in = _concat_inputs(in_maps)
    out = _run_concat(concat_in)
    return _assemble(out[0])
